# revision 6
# baseline (speedup 1.0000x reference)
"""Trainium2 Bass kernel for nn_AdaptiveBilinear.

Reference computation (per batch item b, L=2048, D=512):
    a1  = softmax(x1 @ x1^T)        # (L, L)
    a2  = softmax(x2 @ x2^T)        # (L, L)
    x12 = x1 @ x2^T                 # (L, L)
    out = a1 @ x12 @ a2^T           # (L, L)

Key restructure (exact, by matmul associativity):
    out = (a1 @ x1) @ (a2 @ x2)^T = y1 @ y2^T

so each branch is a self-attention with V=X (5*L^2*D FLOPs total instead of
2*L^3 + 3*L^2*D).

Sharding: batch=8 over the 8 NeuronCores, one batch item per core; the
program is pure SPMD with no collectives.

Per-core algorithm (all matmuls bf16 with f32 PSUM accumulation):
    xT = ddma-transpose(x)                       # [D, L]
    diag[i] = sum_d x[i,d]^2                     # ones-lhsT matmul over squares
    S[j,i] = sum_d xT[d,j] xT[d,i]               # symmetric
    PT[j,i] = exp(S[j,i] - diag[i])              # transposed unnormalized softmax
                                                 # (exact: any per-column constant;
                                                 # diag is the row max here so no
                                                 # overflow, see proto.py)
    sums[i] = sum_j PT[j,i]                      # ones-lhsT matmul, interleaved
    rs = exp(-ln(sums))                          # ScalarE; avoids slow DVE divide
    uT[d,i] = sum_j x[j,d] PT[j,i]               # natural-layout lhsT; no P transposes
    yT[d,i] = uT[d,i] * rs[i]                    # row-broadcast tile (GpSimd bcast)
    out[i,l] = sum_d y1T[d,i] y2T[d,l]
"""

import numpy as np

import concourse.bass as bass
import concourse.mybir as mybir
import concourse.tile as tile
from concourse import bacc, bass_utils

F32 = mybir.dt.float32
BF16 = mybir.dt.bfloat16
EXP = mybir.ActivationFunctionType.Exp
LN = mybir.ActivationFunctionType.Ln

L = 2048          # sequence length per batch item
D = 512           # feature dim
NB = L // 128     # 16 row blocks
DC = D // 128     # 4 contraction chunks of 128
NC = L // 512     # 4 moving-free chunks of 512
NH = L // 1024    # 2 exp/sub chunks of 1024 per row block
N_CORES = 8


def _build_branch(nc, tc, bi, sb, x_d, yT, ones_col):
    """One attention branch: x (DRAM) -> yT [128, DC, L] bf16 (SBUF)."""
    xb = sb["xb"].tile([128, NB, D], BF16, tag="xb", name=f"xb{bi}")
    xT = sb["xt"].tile([128, DC, L], BF16, tag="xT", name=f"xT{bi}")
    PT = sb["pt"].tile([128, NB, L], BF16, tag="PT", name=f"PT{bi}")
    MX = sb["mx"].tile([128, L], BF16, tag="MX", name=f"MX{bi}")
    RS = sb["rs"].tile([128, L], F32, tag="RS", name=f"RS{bi}")
    ndrow = sb["rows"].tile([1, L], BF16, tag="ndrow", name=f"ndrow{bi}")
    rsrow = sb["rows"].tile([1, L], F32, tag="rsrow", name=f"rsrow{bi}")

    # --- load + cast, then DMA-xbar transposes (keep modes batched) ---
    for j in range(NB):
        stg = sb["stage"].tile([128, D], F32, tag="stg", name=f"stg{bi}_{j}")
        nc.sync.dma_start(stg[:], x_d.ap()[j * 128:(j + 1) * 128, :])
        nc.vector.tensor_copy(xb[:, j, :], stg[:])
    for j in range(NB):
        nc.sync.dma_start_transpose(
            xT[:, :, j * 128:(j + 1) * 128], xb[:, j, :])

    # --- diag[i] = sum_d x[i,d]^2, as a [1, L] row -> MX broadcast tile ---
    with tc.tile_pool(name=f"ps_nd{bi}", bufs=1, space="PSUM") as ps_nd:
        nd_ps = ps_nd.tile([1, L], F32, tag="nd", name=f"nd{bi}")
        for c in range(DC):
            for n in range(NC):
                sq = sb["work"].tile([128, 512], BF16, tag="sq",
                                     name=f"sq{bi}_{c}_{n}")
                nc.vector.tensor_mul(
                    sq[:], xT[:, c, n * 512:(n + 1) * 512],
                    xT[:, c, n * 512:(n + 1) * 512])
                nc.tensor.matmul(
                    nd_ps[:, n * 512:(n + 1) * 512],
                    ones_col[:], sq[:],
                    start=(c == 0), stop=(c == DC - 1),
                )
        nc.scalar.copy(ndrow[:], nd_ps[:])          # f32 PSUM -> bf16 row
    nc.gpsimd.partition_broadcast(MX[:], ndrow[:])

    # --- S chunks + stabilizer subtract + exp + interleaved column sums ---
    with (
        tc.tile_pool(name=f"ps_s{bi}", bufs=2, space="PSUM") as ps_s,
        tc.tile_pool(name=f"ps_sum{bi}", bufs=1, space="PSUM") as ps_sum,
    ):
        sums_ps = ps_sum.tile([1, L], F32, tag="sums", name=f"sums{bi}")
        for j in range(NB):
            for h in range(NH):
                sps = ps_s.tile([128, 1024], F32, tag="S", name=f"S{bi}_{j}_{h}")
                for c in range(DC):
                    for v in range(2):
                        n = 2 * h + v
                        nc.tensor.matmul(
                            sps[:, v * 512:(v + 1) * 512],
                            xT[:, c, j * 128:(j + 1) * 128],
                            xT[:, c, n * 512:(n + 1) * 512],
                            start=(c == 0), stop=(c == DC - 1),
                        )
                nc.vector.tensor_sub(
                    sps[:], sps[:], MX[:, h * 1024:(h + 1) * 1024])
                nc.scalar.activation(
                    PT[:, j, h * 1024:(h + 1) * 1024], sps[:], EXP)
                for v in range(2):
                    n = 2 * h + v
                    nc.tensor.matmul(
                        sums_ps[:, n * 512:(n + 1) * 512],
                        ones_col[:],
                        PT[:, j, n * 512:(n + 1) * 512],
                        start=(j == 0), stop=(j == NB - 1),
                    )
        # rs = exp(-ln(sums))  (1/x on ScalarE; DVE reciprocal is ~13us/row)
        nc.scalar.activation(rsrow[:], sums_ps[:], LN)
        nc.scalar.activation(rsrow[:], rsrow[:], EXP, scale=-1.0)
    nc.gpsimd.partition_broadcast(RS[:], rsrow[:])

    # --- uT[d,i] = sum_j x[j,d] PT[j,i]; yT = uT * RS ---
    with tc.tile_pool(name=f"ps_u{bi}", bufs=8, space="PSUM") as ps_u:
        for c in range(DC):
            ups = [ps_u.tile([128, 512], F32, tag="u", name=f"u{bi}_{c}_{n}")
                   for n in range(NC)]
            for j in range(NB):
                for n in range(NC):
                    nc.tensor.matmul(
                        ups[n][:],
                        xb[:, j, c * 128:(c + 1) * 128],
                        PT[:, j, n * 512:(n + 1) * 512],
                        start=(j == 0), stop=(j == NB - 1),
                    )
            for n in range(NC):
                nc.vector.tensor_mul(
                    yT[:, c, n * 512:(n + 1) * 512],
                    ups[n][:],
                    RS[:, n * 512:(n + 1) * 512],
                )


def build_nc():
    nc = bacc.Bacc("TRN2", target_bir_lowering=False, debug=False,
                   num_devices=N_CORES)
    x1_d = nc.dram_tensor("x1", [L, D], F32, kind="ExternalInput")
    x2_d = nc.dram_tensor("x2", [L, D], F32, kind="ExternalInput")
    out_d = nc.dram_tensor("out", [L, L], F32, kind="ExternalOutput")

    with tile.TileContext(nc) as tc:
        with (
            tc.tile_pool(name="const", bufs=1) as constp,
            tc.tile_pool(name="ypool", bufs=1) as ypool,
            tc.tile_pool(name="xbp", bufs=2) as xbp,
            tc.tile_pool(name="xtp", bufs=1) as xtp,
            tc.tile_pool(name="ptp", bufs=1) as ptp,
            tc.tile_pool(name="mxp", bufs=2) as mxp,
            tc.tile_pool(name="rsp", bufs=2) as rsp,
            tc.tile_pool(name="rows", bufs=1) as rows,
            tc.tile_pool(name="stage", bufs=3) as stage,
            tc.tile_pool(name="work", bufs=2) as work,
            tc.tile_pool(name="osbp", bufs=3) as osbp,
        ):
            ones_col = constp.tile([128, 1], BF16, tag="ones_col")
            nc.gpsimd.memset(ones_col[:], 1.0)

            y1T = ypool.tile([128, DC, L], BF16, tag="y1T")
            y2T = ypool.tile([128, DC, L], BF16, tag="y2T")

            sb = {"xb": xbp, "xt": xtp, "pt": ptp, "mx": mxp, "rs": rsp,
                  "rows": rows, "stage": stage, "work": work}
            _build_branch(nc, tc, 1, sb, x1_d, y1T, ones_col)
            _build_branch(nc, tc, 2, sb, x2_d, y2T, ones_col)

            # --- out[i,l] = sum_d y1T[d,i] y2T[d,l] ---
            with tc.tile_pool(name="ps_o", bufs=3, space="PSUM") as ps_o:
                for i in range(NB):
                    for h in range(NH):
                        ops = ps_o.tile([128, 1024], F32, tag="o",
                                        name=f"o_{i}_{h}")
                        for c in range(DC):
                            for v in range(2):
                                n = 2 * h + v
                                nc.tensor.matmul(
                                    ops[:, v * 512:(v + 1) * 512],
                                    y1T[:, c, i * 128:(i + 1) * 128],
                                    y2T[:, c, n * 512:(n + 1) * 512],
                                    start=(c == 0), stop=(c == DC - 1),
                                )
                        osb = osbp.tile([128, 1024], F32, tag="osb",
                                        name=f"osb_{i}_{h}")
                        if h % 2 == 0:
                            nc.scalar.copy(osb[:], ops[:])
                        else:
                            nc.vector.tensor_copy(osb[:], ops[:])
                        nc.sync.dma_start(
                            out_d.ap()[i * 128:(i + 1) * 128,
                                       h * 1024:(h + 1) * 1024],
                            osb[:])

    nc.compile()
    return nc


_NC_CACHE = None


def _get_nc():
    global _NC_CACHE
    if _NC_CACHE is None:
        _NC_CACHE = build_nc()
    return _NC_CACHE


def kernel(x1: np.ndarray, x2: np.ndarray) -> np.ndarray:
    """Full inputs (8, 2048, 512) f32 -> full output (8, 2048, 2048) f32."""
    assert x1.shape == (N_CORES, L, D) and x2.shape == (N_CORES, L, D)
    nc = _get_nc()
    in_maps = [
        {
            "x1": np.ascontiguousarray(np.asarray(x1[b], dtype=np.float32)),
            "x2": np.ascontiguousarray(np.asarray(x2[b], dtype=np.float32)),
        }
        for b in range(N_CORES)
    ]
    res = bass_utils.run_bass_kernel_spmd(nc, in_maps, core_ids=list(range(N_CORES)))
    out = np.stack([res.results[b]["out"] for b in range(N_CORES)], axis=0)
    return out.astype(np.float32, copy=False)


if __name__ == "__main__":
    rng = np.random.default_rng(0)
    x1 = rng.standard_normal((N_CORES, L, D), dtype=np.float32)
    x2 = rng.standard_normal((N_CORES, L, D), dtype=np.float32)
    out = kernel(x1=x1, x2=x2)
    print("kernel output:", out.shape, out.dtype)


# revision 7
# speedup vs baseline: 1.0654x; 1.0654x over previous
"""Trainium2 Bass kernel for nn_AdaptiveBilinear.

Reference computation (per batch item b, L=2048, D=512):
    a1  = softmax(x1 @ x1^T)        # (L, L)
    a2  = softmax(x2 @ x2^T)        # (L, L)
    x12 = x1 @ x2^T                 # (L, L)
    out = a1 @ x12 @ a2^T           # (L, L)

Key restructure (exact, by matmul associativity):
    out = (a1 @ x1) @ (a2 @ x2)^T = y1 @ y2^T

so each branch is a self-attention with V=X (5*L^2*D FLOPs total instead of
2*L^3 + 3*L^2*D).

Sharding: batch=8 over the 8 NeuronCores, one batch item per core; the
program is pure SPMD with no collectives.

Per-core algorithm (all matmuls bf16 with f32 PSUM accumulation):
    diag[i] = sum_d x[i,d]^2          # ScalarE Square w/ accumulate, from f32 stage
    xT = dma-xbar-transpose(x_bf16)   # [D, L]
    S[j,i] = sum_d xT[d,j] xT[d,i]    # symmetric
    PT[j,i] = exp(S[j,i] - diag[i])   # transposed unnormalized softmax; valid
                                      # for any per-column constant, and diag is
                                      # the row max here so exp never overflows
    sums[i] = sum_j PT[j,i]           # ones-lhsT matmuls
    rs = exp(-ln(sums))               # 1/x on ScalarE (DVE reciprocal is ~13us/row)
    uT[d,i] = sum_j x[j,d] PT[j,i]    # natural-layout lhsT; no P transposes
    yT[d,i] = uT[d,i] * rs[i]         # row-broadcast tile (GpSimd partition bcast)
    out[i,l] = sum_d y1T[d,i] y2T[d,l]
"""

import numpy as np

import concourse.bass as bass
import concourse.mybir as mybir
import concourse.tile as tile
from concourse import bacc, bass_utils
from concourse.masks import make_identity

F32 = mybir.dt.float32
BF16 = mybir.dt.bfloat16
EXP = mybir.ActivationFunctionType.Exp
LN = mybir.ActivationFunctionType.Ln
SQUARE = mybir.ActivationFunctionType.Square

L = 2048          # sequence length per batch item
D = 512           # feature dim
NB = L // 128     # 16 row blocks
DC = D // 128     # 4 contraction chunks of 128
NC = L // 512     # 4 moving-free chunks of 512
NH = L // 1024    # 2 exp/sub chunks of 1024 per row block
N_CORES = 8


def _build_branch(nc, tc, bi, sb, x_d, yT, ones_col, ident):
    """One attention branch: x (DRAM) -> yT [128, DC, L] bf16 (SBUF)."""
    xb = sb["xb"].tile([128, NB, D], BF16, tag="xb", name=f"xb{bi}")
    xT = sb["xt"].tile([128, DC, L], BF16, tag="xT", name=f"xT{bi}")
    PT = sb["pt"].tile([128, NB, L], BF16, tag="PT", name=f"PT{bi}")
    MX = sb["mx"].tile([128, L], BF16, tag="MX", name=f"MX{bi}")
    RS = sb["rs"].tile([128, L], F32, tag="RS", name=f"RS{bi}")
    diagcols = sb["rows"].tile([128, NB], F32, tag="diagcols", name=f"dc{bi}")
    diagT = sb["rows"].tile([NB, 128], BF16, tag="diagT", name=f"dT{bi}")
    ndrow = sb["rows"].tile([1, L], BF16, tag="ndrow", name=f"ndrow{bi}")
    rsrow = sb["rows"].tile([1, L], F32, tag="rsrow", name=f"rsrow{bi}")

    # --- load; diag accumulation on ScalarE; cast; xbar transposes ---
    for j in range(NB):
        stg = sb["stage"].tile([128, D], F32, tag="stg", name=f"stg{bi}_{j}")
        nc.sync.dma_start(stg[:], x_d.ap()[j * 128:(j + 1) * 128, :])
        junk = sb["work"].tile([128, D], BF16, tag="junk", name=f"jk{bi}_{j}")
        nc.scalar.activation(junk[:], stg[:], SQUARE,
                             accum_out=diagcols[:, j:j + 1])
        nc.vector.tensor_copy(xb[:, j, :], stg[:])
    for j in range(NB):
        nc.sync.dma_start_transpose(
            xT[:, :, j * 128:(j + 1) * 128], xb[:, j, :])

    # diag row: PE-transpose the accumulated columns, linearize, broadcast.
    with tc.tile_pool(name=f"ps_m{bi}", bufs=1, space="PSUM") as ps_m:
        dtp = ps_m.tile([NB, 128], F32, tag="dtp", name=f"dtp{bi}")
        nc.tensor.transpose(dtp[:], diagcols[:], ident[:])
        nc.vector.tensor_copy(diagT[:], dtp[:])
    nc.sync.dma_start(ndrow[:], diagT[:])          # [16,128] -> [1,2048]
    nc.gpsimd.partition_broadcast(MX[:], ndrow[:])

    # --- S chunks + stabilizer subtract + exp ---
    with tc.tile_pool(name=f"ps_s{bi}", bufs=3, space="PSUM") as ps_s:
        for j in range(NB):
            sps = [ps_s.tile([128, 1024], F32, tag="S", name=f"S{bi}_{j}_{h}")
                   for h in range(NH)]
            for c in range(DC):
                for h in range(NH):
                    for v in range(2):
                        n = 2 * h + v
                        nc.tensor.matmul(
                            sps[h][:, v * 512:(v + 1) * 512],
                            xT[:, c, j * 128:(j + 1) * 128],
                            xT[:, c, n * 512:(n + 1) * 512],
                            start=(c == 0), stop=(c == DC - 1),
                        )
            for h in range(NH):
                nc.vector.tensor_sub(
                    sps[h][:], sps[h][:], MX[:, h * 1024:(h + 1) * 1024])
                nc.scalar.activation(
                    PT[:, j, h * 1024:(h + 1) * 1024], sps[h][:], EXP)

    # --- column sums of PT (= softmax row sums); rs = exp(-ln(sums)) ---
    with tc.tile_pool(name=f"ps_sum{bi}", bufs=1, space="PSUM") as ps_sum:
        sums_ps = ps_sum.tile([1, L], F32, tag="sums", name=f"sums{bi}")
        for j in range(NB):
            for n in range(NC):
                nc.tensor.matmul(
                    sums_ps[:, n * 512:(n + 1) * 512],
                    ones_col[:],
                    PT[:, j, n * 512:(n + 1) * 512],
                    start=(j == 0), stop=(j == NB - 1),
                )
        nc.scalar.activation(rsrow[:], sums_ps[:], LN)
    nc.scalar.activation(rsrow[:], rsrow[:], EXP, scale=-1.0)
    nc.gpsimd.partition_broadcast(RS[:], rsrow[:])

    # --- uT[d,i] = sum_j x[j,d] PT[j,i]; yT = uT * RS ---
    with tc.tile_pool(name=f"ps_u{bi}", bufs=8, space="PSUM") as ps_u:
        for c in range(DC):
            ups = [ps_u.tile([128, 512], F32, tag="u", name=f"u{bi}_{c}_{n}")
                   for n in range(NC)]
            for j in range(NB):
                for n in range(NC):
                    nc.tensor.matmul(
                        ups[n][:],
                        xb[:, j, c * 128:(c + 1) * 128],
                        PT[:, j, n * 512:(n + 1) * 512],
                        start=(j == 0), stop=(j == NB - 1),
                    )
            for n in range(NC):
                nc.vector.tensor_mul(
                    yT[:, c, n * 512:(n + 1) * 512],
                    ups[n][:],
                    RS[:, n * 512:(n + 1) * 512],
                )


def build_nc():
    nc = bacc.Bacc("TRN2", target_bir_lowering=False, debug=False,
                   num_devices=N_CORES)
    x1_d = nc.dram_tensor("x1", [L, D], F32, kind="ExternalInput")
    x2_d = nc.dram_tensor("x2", [L, D], F32, kind="ExternalInput")
    out_d = nc.dram_tensor("out", [L, L], F32, kind="ExternalOutput")

    with tile.TileContext(nc) as tc:
        with (
            tc.tile_pool(name="const", bufs=1) as constp,
            tc.tile_pool(name="ypool", bufs=1) as ypool,
            tc.tile_pool(name="xbp", bufs=2) as xbp,
            tc.tile_pool(name="xtp", bufs=1) as xtp,
            tc.tile_pool(name="ptp", bufs=1) as ptp,
            tc.tile_pool(name="mxp", bufs=2) as mxp,
            tc.tile_pool(name="rsp", bufs=2) as rsp,
            tc.tile_pool(name="rows", bufs=1) as rows,
            tc.tile_pool(name="stage", bufs=3) as stage,
            tc.tile_pool(name="work", bufs=2) as work,
            tc.tile_pool(name="osbp", bufs=3) as osbp,
        ):
            ones_col = constp.tile([128, 1], BF16, tag="ones_col")
            nc.gpsimd.memset(ones_col[:], 1.0)
            ident = constp.tile([128, 128], F32, tag="ident")
            make_identity(nc, ident[:])

            y1T = ypool.tile([128, DC, L], BF16, tag="y1T")
            y2T = ypool.tile([128, DC, L], BF16, tag="y2T")

            sb = {"xb": xbp, "xt": xtp, "pt": ptp, "mx": mxp, "rs": rsp,
                  "rows": rows, "stage": stage, "work": work}
            _build_branch(nc, tc, 1, sb, x1_d, y1T, ones_col, ident)
            _build_branch(nc, tc, 2, sb, x2_d, y2T, ones_col, ident)

            # --- out[i,l] = sum_d y1T[d,i] y2T[d,l] ---
            with tc.tile_pool(name="ps_o", bufs=2, space="PSUM") as ps_o:
                for i in range(NB):
                    ops = ps_o.tile([128, L], F32, tag="o", name=f"o_{i}")
                    for c in range(DC):
                        for n in range(NC):
                            nc.tensor.matmul(
                                ops[:, n * 512:(n + 1) * 512],
                                y1T[:, c, i * 128:(i + 1) * 128],
                                y2T[:, c, n * 512:(n + 1) * 512],
                                start=(c == 0), stop=(c == DC - 1),
                            )
                    for h in range(NH):
                        osb = osbp.tile([128, 1024], F32, tag="osb",
                                        name=f"osb_{i}_{h}")
                        if h % 2 == 0:
                            nc.scalar.copy(osb[:], ops[:, h * 1024:(h + 1) * 1024])
                        else:
                            nc.vector.tensor_copy(
                                osb[:], ops[:, h * 1024:(h + 1) * 1024])
                        nc.sync.dma_start(
                            out_d.ap()[i * 128:(i + 1) * 128,
                                       h * 1024:(h + 1) * 1024],
                            osb[:])

    nc.compile()
    return nc


_NC_CACHE = None


def _get_nc():
    global _NC_CACHE
    if _NC_CACHE is None:
        _NC_CACHE = build_nc()
    return _NC_CACHE


def kernel(x1: np.ndarray, x2: np.ndarray) -> np.ndarray:
    """Full inputs (8, 2048, 512) f32 -> full output (8, 2048, 2048) f32."""
    assert x1.shape == (N_CORES, L, D) and x2.shape == (N_CORES, L, D)
    nc = _get_nc()
    in_maps = [
        {
            "x1": np.ascontiguousarray(np.asarray(x1[b], dtype=np.float32)),
            "x2": np.ascontiguousarray(np.asarray(x2[b], dtype=np.float32)),
        }
        for b in range(N_CORES)
    ]
    res = bass_utils.run_bass_kernel_spmd(nc, in_maps, core_ids=list(range(N_CORES)))
    out = np.stack([res.results[b]["out"] for b in range(N_CORES)], axis=0)
    return out.astype(np.float32, copy=False)


if __name__ == "__main__":
    rng = np.random.default_rng(0)
    x1 = rng.standard_normal((N_CORES, L, D), dtype=np.float32)
    x2 = rng.standard_normal((N_CORES, L, D), dtype=np.float32)
    out = kernel(x1=x1, x2=x2)
    print("kernel output:", out.shape, out.dtype)
